# revision 45
# baseline (speedup 1.0000x reference)
"""Trainium2 Bass kernel for nn_MultiHeadAttention_68865505624655.

Strategy (head parallelism, 8 cores x 2 heads; all-bf16 matmul datapath):
  The reference's reshape(B,-1,T,H) mixes time/channel dims. For head h the
  per-head matrices are exactly reinterpretations of the compacted projection
  output Y_h = X @ W[h::16].T (shape (3072, 64)):
      Q_h^T (xi, t2)  == Y_h viewed as (64, 3072)   (same linear memory!)
      K_h^T (xi, t2)  == same
      V_h  (t2', xi)  == transpose of that view     (needs a real transpose)
  Each core:
    1. fused QKV projection for its 2 heads (bf16 in, fp32 psum): Y6 =
       X @ [Wq1|Wk1|Wv1|Wq2|Wk2|Wv2]^T, + bias, ONE write DMA per t-block to
       per-chunk DRAM scratch (separate tensors so chunk readbacks overlap
       the projection; the last chunk is small since it gates the serial
       projection->readback->transpose tail).
    2. reads back K^T/V^T as (64,3072) contiguous bf16 views; V is PE-
       transposed per c-tile (both heads in one 128-wide transpose) into a
       padded [V1|1|0..|V2|1|0..] (128,256) layout: the AV stationary
       operand is then M=128 with a built-in ones column, so out row 64
       accumulates the softmax denominator for free.
    3. r-loop attention, SEG=2 (one c-tile's TWO heads per PSUM slot, so the
       energy pair runs concurrently in disjoint PE row groups; 3 ep bufs +
       2 AV accumulators = 8 PSUM banks exactly). The softmax exp runs on
       BOTH non-tensor engines every batch: ScalarE true exp for head 1, and
       DVE computes head 2 as int16(S*128/ln2 + 16250) whose raw bits ARE
       bf16 exp(S) (Schraudolph; num/den share weights so the ~3% elementwise
       error cancels to ~5e-3 end-to-end). The AV matmul reads the int16
       tile via AP.bitcast(bf16). No max-subtraction (energies bounded).
    4. writes per-head [out^T; Sigma] (65,3072) f32 tiles per core.
  Host: divide rows 0:64 by row 64, interleave heads into (T,D), gamma*out+x.
  Toolchain workarounds: _split_multiwaits (this walrus allows one sync wait
  per instruction) and _install_ntff_shim (axon NTFF profiling hook).
"""

import sys

if "/opt/trn_rl_repo" not in sys.path:
    sys.path.insert(0, "/opt/trn_rl_repo")

import numpy as np


def _install_ntff_shim():
    """concourse.bass_utils under axon imports antenv.axon_hooks when
    tracing is requested; this image's antenv lacks that submodule.
    Register an equivalent shim (backed by the boot image's ctypes NTFF
    driver) so BASS_TRACE=1 profiles instead of crashing."""
    import types

    if "antenv.axon_hooks" in sys.modules:
        return
    mod = types.ModuleType("antenv.axon_hooks")
    cell = {}

    def get_axon_ntff_profile_hook():
        if "h" not in cell:
            try:
                from trn_agent_boot.trn_boot import _ntff_profile_via_ctypes
                cell["h"] = _ntff_profile_via_ctypes("/opt/axon/libaxon_pjrt.so")
            except Exception:
                cell["h"] = None
        return cell["h"]

    def set_axon_ntff_profile_hook(h):
        cell["h"] = h

    mod.get_axon_ntff_profile_hook = get_axon_ntff_profile_hook
    mod.set_axon_ntff_profile_hook = set_axon_ntff_profile_hook
    sys.modules["antenv.axon_hooks"] = mod


_install_ntff_shim()

import concourse.bass as bass
import concourse.mybir as mybir
import concourse.tile as tile
from concourse.bass import ds, ts
from concourse.masks import make_identity

F32 = mybir.dt.float32
BF16 = mybir.dt.bfloat16

T = 3072          # sequence length (and t2 size)
D = 1024          # model dim
H = 16            # heads
NCORE = 8
EG = 64           # channel groups per head (columns of Y_h)
XI = 64           # "feature" dim of the quirky attention (t // 48)
NKT = D // 128    # 8 contraction tiles for the projection
NTB = T // 128    # 24 t-blocks / c-tiles
RCH = 512         # r-chunk (free dim of energy/AV matmuls)
NR = T // RCH     # 6 r-chunks
W6 = 6 * EG       # 384 fused projection output columns
NCHUNK = 4        # kt/vt readback chunks (fired as t-blocks complete)
# uneven chunk sizes (in 128-t blocks): the LAST chunk gates the serial
# projection->readback->transpose tail, so make it small
CHB = [9, 9, 3, 3]                    # blocks per chunk
CB0 = [0, 9, 18, 21]                  # first block of each chunk
XO = [0, 24, 48, 56]                  # first xi row of each chunk
XN = [24, 24, 8, 8]                   # xi rows per chunk (t-rows / 48)
# Schraudolph exp-as-int16-bitcast constants for the DVE softmax path:
# bf16(bitcast(int16(S*C16 + D16))) ~= exp(S) to ~3% elementwise; the
# softmax numerator/denominator share the same approximated weights so the
# error largely cancels (measured end-to-end rel err ~5e-3).
C16 = 184.66496523378264    # 128 / ln(2)
D16 = 16250.0               # 127*128 - 6 (balanced linear-interp error)


def _split_multiwaits(nc):
    """This toolchain's walrus accepts at most ONE sync wait per
    instruction (setupSyncWait: 'Too many sync wait commands'), but Tile
    attaches several. Hoist all but the last wait of each instruction onto
    same-engine NoOps inserted right before it — semantically identical
    (sem-ge waits executed in sequence)."""
    n = 0
    for fn in nc.m.functions:
        for bb in fn.blocks:
            insts = list(bb.instructions)
            out = []
            changed = False
            for inst in insts:
                si = inst.sync_info
                if si is not None and len(si.on_wait) > 1:
                    waits = list(si.on_wait)
                    for w in waits[:-1]:
                        n += 1
                        out.append(mybir.InstNoOp(
                            name=f"I-splitwait-{n}",
                            ins=[], outs=[], engine=inst.engine,
                            sync_info=mybir.SyncInfo(on_wait=[w], on_update=[]),
                        ))
                    inst.sync_info = mybir.SyncInfo(
                        on_wait=[waits[-1]], on_update=list(si.on_update)
                    )
                    changed = True
                out.append(inst)
            if changed:
                bb.instructions = out
    return n


def build_program():
    nc = bass.Bass()

    xT = nc.dram_tensor("xT", [NTB, 128, NKT, 128], BF16, kind="ExternalInput")
    w6 = nc.dram_tensor("w6", [D, W6], BF16, kind="ExternalInput")
    b6 = nc.dram_tensor("b6", [128, W6], F32, kind="ExternalInput")
    # per-readback-chunk scratch tensors: separate DRAM tensors (not slices
    # of one) so the chunk readback DMAs' dependencies resolve as soon as
    # their own 6 t-blocks are written, letting them overlap the projection.
    # Layout [6(q1,k1,v1,q2,k2,v2), TCH, EG], stream-major: ONE write DMA per
    # t-block (per-ring dma_start dispatch overhead dominates the write path)
    # and contiguous 6KB-per-partition kt/vt readbacks; the strided q reads
    # happen in the attention phase where DMA is otherwise idle.
    y6aC = [nc.dram_tensor(f"y6a{cc}", [6, 128 * CHB[cc], EG], BF16,
                           kind="Internal")
            for cc in range(NCHUNK)]
    outT = nc.dram_tensor("outT", [2, XI + 1, T], F32, kind="ExternalOutput")

    with tile.TileContext(nc) as tc:
        with tc.tile_pool(name="const", bufs=1) as constp:
            w6_sb = constp.tile([128, NKT, W6], BF16)
            w6v = w6[:, :].rearrange("(k p) n -> k p n", p=128)
            for k in range(NKT):
                nc.scalar.dma_start(out=w6_sb[:, k, :], in_=w6v[k, :, :])
            b6_sb = constp.tile([128, W6], F32)
            nc.scalar.dma_start(out=b6_sb, in_=b6[:, :])
            # touch the Exp table at program start so ACT_TABLE_LOAD happens
            # during the (ACT-idle) projection phase, not at the first real
            # softmax exp
            warm = constp.tile([128, 8], F32)
            nc.scalar.activation(warm, b6_sb[:, 0:8],
                                 mybir.ActivationFunctionType.Exp)
            # full 128x128 identity: one PE transpose flips BOTH heads' V^T
            # c-chunks at once (cols 0:64 head1, 64:128 head2)
            ident = constp.tile([128, 128], BF16)
            make_identity(nc, ident)
            ones24 = constp.tile([128, NTB], F32)
            nc.gpsimd.memset(ones24, 1.0)
            kt_sb = constp.tile([128, T], BF16)   # rows 0:64 h1 K^T, 64:128 h2
            vt_sb = constp.tile([128, T], BF16)   # rows 0:64 h1 V^T, 64:128 h2
            # AV stationary operands, padded to 128 columns:
            # [:, c, 0:64] = V1_c, [:, c, 64] = 1.0 (denominator row),
            # [:, c, 65:128] = 0, [:, c, 128:192] = V2_c, [192] = 1, rest 0.
            v12x = constp.tile([128, NTB, 256], BF16)
            nc.gpsimd.memset(v12x, 0.0)
            for col in (64, 192):
                nc.vector.tensor_copy(
                    v12x[:, :, col:col + 1].rearrange("p c one -> p (c one)"),
                    ones24,
                )

            # ---------------- projection: Y6 = X @ W6^T + b6 ----------------
            with tc.tile_pool(name="xt", bufs=8) as xtp, \
                 tc.tile_pool(name="psy", bufs=4, space="PSUM") as psyp, \
                 tc.tile_pool(name="ysb", bufs=6) as ysbp:
                for j in range(NTB):
                    xt = xtp.tile([128, NKT, 128], BF16)
                    nc.sync.dma_start(out=xt, in_=xT[j, :, :, :])
                    psy = psyp.tile([128, W6], F32)
                    for k in range(NKT):
                        nc.tensor.matmul(
                            psy, xt[:, k, :], w6_sb[:, k, :],
                            start=(k == 0), stop=(k == NKT - 1),
                        )
                    psyv = psy.rearrange("p (h z e) -> p h z e", h=2, z=3)
                    b6v = b6_sb.rearrange("p (h z e) -> p h z e", h=2, z=3)
                    ysb = ysbp.tile([128, 2, 3, EG], BF16, name="ysb")
                    nc.vector.tensor_add(ysb, psyv, b6v)
                    jc = sum(1 for b in CB0[1:] if j >= b)
                    jl = j - CB0[jc]
                    # the LAST chunk's writes + readbacks ride the sync ring:
                    # it is empty once the xt loads finish, so the final
                    # write->readback->transpose chain is not stuck behind
                    # the scalar ring's descriptor backlog
                    # the LAST chunk's writes + readbacks ride the sync ring
                    # (empty once the xt loads finish); earlier chunks'
                    # writes alternate scalar/sync so neither ring's
                    # descriptor backlog delays a chunk's last write (which
                    # gates that chunk's readback). The xt prefetch depth
                    # absorbs the occasional sync-ring wait.
                    if jc == NCHUNK - 1:
                        weng = nc.sync
                    else:
                        weng = nc.scalar if j % 2 == 0 else nc.sync
                    weng.dma_start(
                        out=y6aC[jc][:, ts(jl, 128), :].rearrange(
                            "q t e -> t q e"),
                        in_=ysb.rearrange("p h z e -> p (h z) e"),
                    )
                    # fire kt/vt readback for chunk jc right after its last
                    # t-block write -- on the OTHERWISE-IDLE gpsimd software
                    # DGE: a readback trigger waits for its chunk's writes,
                    # and on the scalar ring that wait head-of-line-blocks
                    # every later write trigger (measured: all readbacks
                    # slipped to after the full projection)
                    if jl == CHB[jc] - 1:
                        rows = ds(XO[jc], XN[jc])
                        reng = nc.sync if jc == NCHUNK - 1 else nc.gpsimd
                        for qi, bufap in (
                                (2, vt_sb[0:64, :]),
                                (5, vt_sb[64:128, :])):
                            reng.dma_start(
                                out=bufap.rearrange(
                                    "p (a e) -> p a e", a=48)[rows, :, :],
                                in_=y6aC[jc][qi, :, :].rearrange(
                                    "(xi a) e -> xi a e", xi=XN[jc]),
                            )

            for jc in range(NCHUNK):
                rows = ds(XO[jc], XN[jc])
                for qi, bufap in ((1, kt_sb[0:64, :]), (4, kt_sb[64:128, :])):
                    nc.sync.dma_start(
                        out=bufap.rearrange(
                            "p (a e) -> p a e", a=48)[rows, :, :],
                        in_=y6aC[jc][qi, :, :].rearrange(
                            "(xi a) e -> xi a e", xi=XN[jc]),
                    )

            # ------- V tiles: one 128-wide PE transpose per c-tile ----------
            with tc.tile_pool(name="vtps", bufs=4, space="PSUM") as vtpsp:
                for c in range(NTB):
                    vp = vtpsp.tile([128, 128], BF16)
                    nc.tensor.transpose(vp, vt_sb[:, ts(c, 128)], ident)
                    nc.vector.tensor_copy(v12x[:, c, 0:XI], vp[:, 0:XI])
                    nc.vector.tensor_copy(v12x[:, c, 128:128 + XI],
                                          vp[:, XI:128])

            # --------------------------- attention --------------------------
            # (c-tile, head) pairs are enumerated as g = 2c + head; SEG=2
            # packs one c-tile's TWO heads per PSUM slot, so the energy pair
            # runs concurrently in disjoint PE row groups, and THREE ep bufs
            # (3x2 banks + 2 outp banks = 8) keep a free slot for the next
            # energy batch while BOTH exp engines (ScalarE true exp / DVE
            # Schraudolph) are draining their own in-flight batches.
            SEG = 2
            with tc.tile_pool(name="qt", bufs=2) as qtp, \
                 tc.tile_pool(name="eps", bufs=3, space="PSUM") as epp, \
                 tc.tile_pool(name="ex", bufs=2) as expool, \
                 tc.tile_pool(name="exi", bufs=2) as exipool, \
                 tc.tile_pool(name="outp", bufs=1, space="PSUM") as outpp, \
                 tc.tile_pool(name="osb", bufs=4) as osbp:
                for r in range(NR):
                    qt = qtp.tile([128, RCH], BF16)
                    for qi, row0 in ((0, 0), (3, 64)):
                        for cc in range(NCHUNK):
                            nc.sync.dma_start(
                                out=qt[row0 + XO[cc]:row0 + XO[cc] + XN[cc],
                                       :].rearrange(
                                    "p (a e) -> p a e", a=RCH // EG),
                                in_=y6aC[cc][qi, :, :].rearrange(
                                    "(xi a) e -> xi a e", xi=XN[cc])[
                                    :, ds(r * (RCH // EG), RCH // EG), :],
                            )
                    outp1 = outpp.tile([128, RCH], F32)
                    outp2 = outpp.tile([128, RCH], F32)
                    ep = None
                    pend = []
                    for g in range(2 * NTB):
                        c, hl = divmod(g, 2)
                        s = g % SEG
                        if s == 0:
                            ep = epp.tile([128, SEG * RCH], F32)
                        row0 = hl * 64
                        nc.tensor.matmul(
                            ep[:, ds(s * RCH, RCH)],
                            kt_sb[row0:row0 + 64, ts(c, 128)],
                            qt[row0:row0 + 64, :],
                            start=True, stop=True,
                        )
                        pend.append((hl, c, s))
                        if s == SEG - 1:
                            # both exp engines run on EVERY batch, one
                            # 512-slice each: ScalarE true exp on head 1,
                            # DVE Schraudolph (int16 bitcast-as-bf16) on
                            # head 2 -- halves the exp latency in the
                            # E -> exp -> AV chain and frees the ep PSUM
                            # buffer sooner
                            ex = expool.tile([128, RCH], BF16)
                            nc.scalar.activation(
                                ex, ep[:, ds(0, RCH)],
                                mybir.ActivationFunctionType.Exp
                            )
                            exi = exipool.tile([128, RCH], mybir.dt.int16)
                            nc.vector.tensor_scalar(
                                exi, ep[:, ds(RCH, RCH)], C16, D16,
                                mybir.AluOpType.mult, mybir.AluOpType.add,
                            )
                            srcs = (ex, exi.bitcast(BF16))
                            for phl, pc, ps in pend:
                                outp = outp1 if phl == 0 else outp2
                                nc.tensor.matmul(
                                    outp, v12x[:, pc, ds(phl * 128, 128)],
                                    srcs[ps],
                                    start=(pc == 0), stop=(pc == NTB - 1),
                                )
                            pend = []
                    # drain the two accumulators on DIFFERENT engines so the
                    # next r-chunk's first AV (outp bufs=1) waits half as long
                    osb1 = osbp.tile([XI + 1, RCH], F32, name="osb1")
                    nc.vector.tensor_copy(osb1, outp1[0:XI + 1, :])
                    osb2 = osbp.tile([XI + 1, RCH], F32, name="osb2")
                    nc.scalar.copy(osb2, outp2[0:XI + 1, :])
                    for osb, hl in ((osb1, 0), (osb2, 1)):
                        nc.sync.dma_start(
                            out=outT[hl, :, ts(r, RCH)], in_=osb
                        )
    return nc


def make_in_maps(x, Wq, bq, Wk, bk, Wv, bv):
    import ml_dtypes
    bf16 = ml_dtypes.bfloat16
    X = np.ascontiguousarray(np.asarray(x, dtype=np.float32).reshape(T, D))
    # (NTB, 128, NKT, 128): [j, p, k, t] = X[128j+t, 128k+p] -- every SBUF
    # partition reads one contiguous 2KB run per projection slab DMA
    xTm = np.ascontiguousarray(
        X.reshape(NTB, 128, NKT, 128).transpose(0, 3, 2, 1)
    ).astype(bf16)
    in_maps = []
    for c in range(NCORE):
        wcols, bcols = [], []
        for h in (2 * c, 2 * c + 1):
            for W, b in ((Wq, bq), (Wk, bk), (Wv, bv)):
                wcols.append(np.asarray(W, np.float32)[h::H, :].T)
                bcols.append(np.asarray(b, np.float32)[h::H])
        w6m = np.ascontiguousarray(np.concatenate(wcols, axis=1)).astype(bf16)
        b6m = np.ascontiguousarray(
            np.broadcast_to(np.concatenate(bcols), (128, W6))
        )
        in_maps.append({"xT": xTm, "w6": w6m, "b6": b6m})
    return X, in_maps


def assemble(X, results, gamma):
    O = np.empty((T, EG, H), dtype=np.float32)
    for c in range(NCORE):
        res = results[c]
        for hl in range(2):
            h = 2 * c + hl
            onn = res["outT"][hl][0:XI, :]                # (64, 3072)
            s = res["outT"][hl][XI, :]                    # (3072,)
            O[:, :, h] = (onn / s[None, :]).T
    out = O.reshape(T, D)
    g = np.float32(np.asarray(gamma))
    return (g * out + X).reshape(1, 1, T, D).astype(np.float32)


_PROGRAM = None
last_run_info = {}


def kernel(x, Wq, bq, Wk, bk, Wv, bv, gamma):
    global _PROGRAM
    from concourse import bass_utils

    X, in_maps = make_in_maps(x, Wq, bq, Wk, bk, Wv, bv)
    if _PROGRAM is None:
        _PROGRAM = build_program()
        # required for this toolchain's walrus (1 sync wait per instruction);
        # applied here so CoreSim (which predates these NoOps) can still run
        # the unsplit program from build_program()
        _split_multiwaits(_PROGRAM)
    res = bass_utils.run_bass_kernel_spmd(
        _PROGRAM, in_maps, core_ids=list(range(NCORE))
    )
    last_run_info["exec_time_ns"] = res.exec_time_ns
    last_run_info["trace"] = res.instructions_and_trace
    return assemble(X, res.results, gamma)


# revision 46
# speedup vs baseline: 1.0052x; 1.0052x over previous
"""Trainium2 Bass kernel for nn_MultiHeadAttention_68865505624655.

Strategy (head parallelism, 8 cores x 2 heads; all-bf16 matmul datapath):
  The reference's reshape(B,-1,T,H) mixes time/channel dims. For head h the
  per-head matrices are exactly reinterpretations of the compacted projection
  output Y_h = X @ W[h::16].T (shape (3072, 64)):
      Q_h^T (xi, t2)  == Y_h viewed as (64, 3072)   (same linear memory!)
      K_h^T (xi, t2)  == same
      V_h  (t2', xi)  == transpose of that view     (needs a real transpose)
  Each core:
    1. fused QKV projection for its 2 heads (bf16 in, fp32 psum): Y6 =
       X @ [Wq1|Wk1|Wv1|Wq2|Wk2|Wv2]^T, + bias, ONE write DMA per t-block to
       per-chunk DRAM scratch (separate tensors so chunk readbacks overlap
       the projection; the last chunk is small since it gates the serial
       projection->readback->transpose tail).
    2. reads back K^T/V^T as (64,3072) contiguous bf16 views; V is PE-
       transposed per c-tile (both heads in one 128-wide transpose) into a
       padded [V1|1|0..|V2|1|0..] (128,256) layout: the AV stationary
       operand is then M=128 with a built-in ones column, so out row 64
       accumulates the softmax denominator for free.
    3. r-loop attention, SEG=2 (one c-tile's TWO heads per PSUM slot, so the
       energy pair runs concurrently in disjoint PE row groups; 3 ep bufs +
       2 AV accumulators = 8 PSUM banks exactly). The softmax exp runs on
       BOTH non-tensor engines every batch: ScalarE true exp for head 1, and
       DVE computes head 2 as int16(S*128/ln2 + 16250) whose raw bits ARE
       bf16 exp(S) (Schraudolph; num/den share weights so the ~3% elementwise
       error cancels to ~5e-3 end-to-end). The AV matmul reads the int16
       tile via AP.bitcast(bf16). No max-subtraction (energies bounded).
    4. writes per-head [out^T; Sigma] (65,3072) f32 tiles per core.
  Host: divide rows 0:64 by row 64, interleave heads into (T,D), gamma*out+x.
  Toolchain workarounds: _split_multiwaits (this walrus allows one sync wait
  per instruction) and _install_ntff_shim (axon NTFF profiling hook).
"""

import sys

if "/opt/trn_rl_repo" not in sys.path:
    sys.path.insert(0, "/opt/trn_rl_repo")

import numpy as np


def _install_ntff_shim():
    """concourse.bass_utils under axon imports antenv.axon_hooks when
    tracing is requested; this image's antenv lacks that submodule.
    Register an equivalent shim (backed by the boot image's ctypes NTFF
    driver) so BASS_TRACE=1 profiles instead of crashing."""
    import types

    if "antenv.axon_hooks" in sys.modules:
        return
    mod = types.ModuleType("antenv.axon_hooks")
    cell = {}

    def get_axon_ntff_profile_hook():
        if "h" not in cell:
            try:
                from trn_agent_boot.trn_boot import _ntff_profile_via_ctypes
                cell["h"] = _ntff_profile_via_ctypes("/opt/axon/libaxon_pjrt.so")
            except Exception:
                cell["h"] = None
        return cell["h"]

    def set_axon_ntff_profile_hook(h):
        cell["h"] = h

    mod.get_axon_ntff_profile_hook = get_axon_ntff_profile_hook
    mod.set_axon_ntff_profile_hook = set_axon_ntff_profile_hook
    sys.modules["antenv.axon_hooks"] = mod


_install_ntff_shim()

import concourse.bass as bass
import concourse.mybir as mybir
import concourse.tile as tile
from concourse.bass import ds, ts
from concourse.masks import make_identity

F32 = mybir.dt.float32
BF16 = mybir.dt.bfloat16

T = 3072          # sequence length (and t2 size)
D = 1024          # model dim
H = 16            # heads
NCORE = 8
EG = 64           # channel groups per head (columns of Y_h)
XI = 64           # "feature" dim of the quirky attention (t // 48)
NKT = D // 128    # 8 contraction tiles for the projection
NTB = T // 128    # 24 t-blocks / c-tiles
RCH = 512         # r-chunk (free dim of energy/AV matmuls)
NR = T // RCH     # 6 r-chunks
W6 = 6 * EG       # 384 fused projection output columns
NCHUNK = 4        # kt/vt readback chunks (fired as t-blocks complete)
# uneven chunk sizes (in 128-t blocks): the LAST chunk gates the serial
# projection->readback->transpose tail, so make it small
CHB = [9, 9, 3, 3]                    # blocks per chunk
CB0 = [0, 9, 18, 21]                  # first block of each chunk
XO = [0, 24, 48, 56]                  # first xi row of each chunk
XN = [24, 24, 8, 8]                   # xi rows per chunk (t-rows / 48)
# Schraudolph exp-as-int16-bitcast constants for the DVE softmax path:
# bf16(bitcast(int16(S*C16 + D16))) ~= exp(S) to ~3% elementwise; the
# softmax numerator/denominator share the same approximated weights so the
# error largely cancels (measured end-to-end rel err ~5e-3).
C16 = 184.66496523378264    # 128 / ln(2)
D16 = 16250.0               # 127*128 - 6 (balanced linear-interp error)


def _split_multiwaits(nc):
    """This toolchain's walrus accepts at most ONE sync wait per
    instruction (setupSyncWait: 'Too many sync wait commands'), but Tile
    attaches several. Hoist all but the last wait of each instruction onto
    same-engine NoOps inserted right before it — semantically identical
    (sem-ge waits executed in sequence)."""
    n = 0
    for fn in nc.m.functions:
        for bb in fn.blocks:
            insts = list(bb.instructions)
            out = []
            changed = False
            for inst in insts:
                si = inst.sync_info
                if si is not None and len(si.on_wait) > 1:
                    waits = list(si.on_wait)
                    for w in waits[:-1]:
                        n += 1
                        out.append(mybir.InstNoOp(
                            name=f"I-splitwait-{n}",
                            ins=[], outs=[], engine=inst.engine,
                            sync_info=mybir.SyncInfo(on_wait=[w], on_update=[]),
                        ))
                    inst.sync_info = mybir.SyncInfo(
                        on_wait=[waits[-1]], on_update=list(si.on_update)
                    )
                    changed = True
                out.append(inst)
            if changed:
                bb.instructions = out
    return n


def build_program():
    nc = bass.Bass()

    xT = nc.dram_tensor("xT", [NTB, 128, NKT, 128], BF16, kind="ExternalInput")
    w6 = nc.dram_tensor("w6", [D, W6], BF16, kind="ExternalInput")
    b6 = nc.dram_tensor("b6", [128, W6], F32, kind="ExternalInput")
    # per-readback-chunk scratch tensors: separate DRAM tensors (not slices
    # of one) so the chunk readback DMAs' dependencies resolve as soon as
    # their own 6 t-blocks are written, letting them overlap the projection.
    # Layout [6(q1,k1,v1,q2,k2,v2), TCH, EG], stream-major: ONE write DMA per
    # t-block (per-ring dma_start dispatch overhead dominates the write path)
    # and contiguous 6KB-per-partition kt/vt readbacks; the strided q reads
    # happen in the attention phase where DMA is otherwise idle.
    y6aC = [nc.dram_tensor(f"y6a{cc}", [6, 128 * CHB[cc], EG], BF16,
                           kind="Internal")
            for cc in range(NCHUNK)]
    outT = nc.dram_tensor("outT", [2, XI + 1, T], F32, kind="ExternalOutput")

    with tile.TileContext(nc) as tc:
        with tc.tile_pool(name="const", bufs=1) as constp:
            w6_sb = constp.tile([128, NKT, W6], BF16)
            w6v = w6[:, :].rearrange("(k p) n -> k p n", p=128)
            for k in range(NKT):
                nc.scalar.dma_start(out=w6_sb[:, k, :], in_=w6v[k, :, :])
            b6_sb = constp.tile([128, W6], F32)
            nc.scalar.dma_start(out=b6_sb, in_=b6[:, :])
            # touch the Exp table at program start so ACT_TABLE_LOAD happens
            # during the (ACT-idle) projection phase, not at the first real
            # softmax exp
            warm = constp.tile([128, 8], F32)
            nc.scalar.activation(warm, b6_sb[:, 0:8],
                                 mybir.ActivationFunctionType.Exp)
            # full 128x128 identity: one PE transpose flips BOTH heads' V^T
            # c-chunks at once (cols 0:64 head1, 64:128 head2)
            ident = constp.tile([128, 128], BF16)
            make_identity(nc, ident)
            ones24 = constp.tile([128, NTB], F32)
            nc.gpsimd.memset(ones24, 1.0)
            kt_sb = constp.tile([128, T], BF16)   # rows 0:64 h1 K^T, 64:128 h2
            vt_sb = constp.tile([128, T], BF16)   # rows 0:64 h1 V^T, 64:128 h2
            # AV stationary operands, padded to 128 columns:
            # [:, c, 0:64] = V1_c, [:, c, 64] = 1.0 (denominator row),
            # [:, c, 65:128] = 0, [:, c, 128:192] = V2_c, [192] = 1, rest 0.
            v12x = constp.tile([128, NTB, 256], BF16)
            nc.gpsimd.memset(v12x, 0.0)
            for col in (64, 192):
                nc.vector.tensor_copy(
                    v12x[:, :, col:col + 1].rearrange("p c one -> p (c one)"),
                    ones24,
                )

            # ---------------- projection: Y6 = X @ W6^T + b6 ----------------
            with tc.tile_pool(name="xt", bufs=8) as xtp, \
                 tc.tile_pool(name="psy", bufs=4, space="PSUM") as psyp, \
                 tc.tile_pool(name="ysb", bufs=6) as ysbp:
                for j in range(NTB):
                    xt = xtp.tile([128, NKT, 128], BF16)
                    nc.sync.dma_start(out=xt, in_=xT[j, :, :, :])
                    psy = psyp.tile([128, W6], F32)
                    for k in range(NKT):
                        nc.tensor.matmul(
                            psy, xt[:, k, :], w6_sb[:, k, :],
                            start=(k == 0), stop=(k == NKT - 1),
                        )
                    psyv = psy.rearrange("p (h z e) -> p h z e", h=2, z=3)
                    b6v = b6_sb.rearrange("p (h z e) -> p h z e", h=2, z=3)
                    ysb = ysbp.tile([128, 2, 3, EG], BF16, name="ysb")
                    nc.vector.tensor_add(ysb, psyv, b6v)
                    jc = sum(1 for b in CB0[1:] if j >= b)
                    jl = j - CB0[jc]
                    # the LAST chunk's writes + readbacks ride the sync ring:
                    # it is empty once the xt loads finish, so the final
                    # write->readback->transpose chain is not stuck behind
                    # the scalar ring's descriptor backlog
                    # the LAST chunk's writes + readbacks ride the sync ring:
                    # it is empty once the xt loads finish, so the final
                    # write->readback->transpose chain is not stuck behind
                    # the scalar ring's descriptor backlog
                    weng = nc.sync if jc == NCHUNK - 1 else nc.scalar
                    weng.dma_start(
                        out=y6aC[jc][:, ts(jl, 128), :].rearrange(
                            "q t e -> t q e"),
                        in_=ysb.rearrange("p h z e -> p (h z) e"),
                    )
                    # fire kt/vt readback for chunk jc right after its last
                    # t-block write -- on the OTHERWISE-IDLE gpsimd software
                    # DGE: a readback trigger waits for its chunk's writes,
                    # and on the scalar ring that wait head-of-line-blocks
                    # every later write trigger (measured: all readbacks
                    # slipped to after the full projection)
                    if jl == CHB[jc] - 1:
                        rows = ds(XO[jc], XN[jc])
                        reng = nc.sync if jc == NCHUNK - 1 else nc.gpsimd
                        for qi, bufap in (
                                (2, vt_sb[0:64, :]),
                                (5, vt_sb[64:128, :])):
                            reng.dma_start(
                                out=bufap.rearrange(
                                    "p (a e) -> p a e", a=48)[rows, :, :],
                                in_=y6aC[jc][qi, :, :].rearrange(
                                    "(xi a) e -> xi a e", xi=XN[jc]),
                            )

            for jc in range(NCHUNK):
                rows = ds(XO[jc], XN[jc])
                for qi, bufap in ((1, kt_sb[0:64, :]), (4, kt_sb[64:128, :])):
                    nc.sync.dma_start(
                        out=bufap.rearrange(
                            "p (a e) -> p a e", a=48)[rows, :, :],
                        in_=y6aC[jc][qi, :, :].rearrange(
                            "(xi a) e -> xi a e", xi=XN[jc]),
                    )

            # ------- V tiles: one 128-wide PE transpose per c-tile ----------
            with tc.tile_pool(name="vtps", bufs=4, space="PSUM") as vtpsp:
                for c in range(NTB):
                    vp = vtpsp.tile([128, 128], BF16)
                    nc.tensor.transpose(vp, vt_sb[:, ts(c, 128)], ident)
                    nc.vector.tensor_copy(v12x[:, c, 0:XI], vp[:, 0:XI])
                    nc.vector.tensor_copy(v12x[:, c, 128:128 + XI],
                                          vp[:, XI:128])

            # --------------------------- attention --------------------------
            # (c-tile, head) pairs are enumerated as g = 2c + head; SEG=2
            # packs one c-tile's TWO heads per PSUM slot, so the energy pair
            # runs concurrently in disjoint PE row groups, and THREE ep bufs
            # (3x2 banks + 2 outp banks = 8) keep a free slot for the next
            # energy batch while BOTH exp engines (ScalarE true exp / DVE
            # Schraudolph) are draining their own in-flight batches.
            SEG = 2
            with tc.tile_pool(name="qt", bufs=2) as qtp, \
                 tc.tile_pool(name="eps", bufs=3, space="PSUM") as epp, \
                 tc.tile_pool(name="ex", bufs=2) as expool, \
                 tc.tile_pool(name="exi", bufs=2) as exipool, \
                 tc.tile_pool(name="outp", bufs=1, space="PSUM") as outpp, \
                 tc.tile_pool(name="osb", bufs=4) as osbp:
                for r in range(NR):
                    qt = qtp.tile([128, RCH], BF16)
                    for qi, row0 in ((0, 0), (3, 64)):
                        for cc in range(NCHUNK):
                            nc.sync.dma_start(
                                out=qt[row0 + XO[cc]:row0 + XO[cc] + XN[cc],
                                       :].rearrange(
                                    "p (a e) -> p a e", a=RCH // EG),
                                in_=y6aC[cc][qi, :, :].rearrange(
                                    "(xi a) e -> xi a e", xi=XN[cc])[
                                    :, ds(r * (RCH // EG), RCH // EG), :],
                            )
                    outp1 = outpp.tile([128, RCH], F32)
                    outp2 = outpp.tile([128, RCH], F32)
                    ep = None
                    pend = []
                    for g in range(2 * NTB):
                        c, hl = divmod(g, 2)
                        s = g % SEG
                        if s == 0:
                            ep = epp.tile([128, SEG * RCH], F32)
                        row0 = hl * 64
                        nc.tensor.matmul(
                            ep[:, ds(s * RCH, RCH)],
                            kt_sb[row0:row0 + 64, ts(c, 128)],
                            qt[row0:row0 + 64, :],
                            start=True, stop=True,
                        )
                        pend.append((hl, c, s))
                        if s == SEG - 1:
                            # both exp engines run on EVERY batch, one
                            # 512-slice each: ScalarE true exp on head 1,
                            # DVE Schraudolph (int16 bitcast-as-bf16) on
                            # head 2 -- halves the exp latency in the
                            # E -> exp -> AV chain and frees the ep PSUM
                            # buffer sooner
                            ex = expool.tile([128, RCH], BF16)
                            nc.scalar.activation(
                                ex, ep[:, ds(0, RCH)],
                                mybir.ActivationFunctionType.Exp
                            )
                            exi = exipool.tile([128, RCH], mybir.dt.int16)
                            nc.vector.tensor_scalar(
                                exi, ep[:, ds(RCH, RCH)], C16, D16,
                                mybir.AluOpType.mult, mybir.AluOpType.add,
                            )
                            srcs = (ex, exi.bitcast(BF16))
                            for phl, pc, ps in pend:
                                outp = outp1 if phl == 0 else outp2
                                nc.tensor.matmul(
                                    outp, v12x[:, pc, ds(phl * 128, 128)],
                                    srcs[ps],
                                    start=(pc == 0), stop=(pc == NTB - 1),
                                )
                            pend = []
                    # drain the two accumulators on DIFFERENT engines so the
                    # next r-chunk's first AV (outp bufs=1) waits half as long
                    osb1 = osbp.tile([XI + 1, RCH], F32, name="osb1")
                    nc.vector.tensor_copy(osb1, outp1[0:XI + 1, :])
                    osb2 = osbp.tile([XI + 1, RCH], F32, name="osb2")
                    nc.scalar.copy(osb2, outp2[0:XI + 1, :])
                    for osb, hl in ((osb1, 0), (osb2, 1)):
                        nc.sync.dma_start(
                            out=outT[hl, :, ts(r, RCH)], in_=osb
                        )
    return nc


def make_in_maps(x, Wq, bq, Wk, bk, Wv, bv):
    import ml_dtypes
    bf16 = ml_dtypes.bfloat16
    X = np.ascontiguousarray(np.asarray(x, dtype=np.float32).reshape(T, D))
    # (NTB, 128, NKT, 128): [j, p, k, t] = X[128j+t, 128k+p] -- every SBUF
    # partition reads one contiguous 2KB run per projection slab DMA
    xTm = np.ascontiguousarray(
        X.reshape(NTB, 128, NKT, 128).transpose(0, 3, 2, 1)
    ).astype(bf16)
    in_maps = []
    for c in range(NCORE):
        wcols, bcols = [], []
        for h in (2 * c, 2 * c + 1):
            for W, b in ((Wq, bq), (Wk, bk), (Wv, bv)):
                wcols.append(np.asarray(W, np.float32)[h::H, :].T)
                bcols.append(np.asarray(b, np.float32)[h::H])
        w6m = np.ascontiguousarray(np.concatenate(wcols, axis=1)).astype(bf16)
        b6m = np.ascontiguousarray(
            np.broadcast_to(np.concatenate(bcols), (128, W6))
        )
        in_maps.append({"xT": xTm, "w6": w6m, "b6": b6m})
    return X, in_maps


def assemble(X, results, gamma):
    O = np.empty((T, EG, H), dtype=np.float32)
    for c in range(NCORE):
        res = results[c]
        for hl in range(2):
            h = 2 * c + hl
            onn = res["outT"][hl][0:XI, :]                # (64, 3072)
            s = res["outT"][hl][XI, :]                    # (3072,)
            O[:, :, h] = (onn / s[None, :]).T
    out = O.reshape(T, D)
    g = np.float32(np.asarray(gamma))
    return (g * out + X).reshape(1, 1, T, D).astype(np.float32)


_PROGRAM = None
last_run_info = {}


def kernel(x, Wq, bq, Wk, bk, Wv, bv, gamma):
    global _PROGRAM
    from concourse import bass_utils

    X, in_maps = make_in_maps(x, Wq, bq, Wk, bk, Wv, bv)
    if _PROGRAM is None:
        _PROGRAM = build_program()
        # required for this toolchain's walrus (1 sync wait per instruction);
        # applied here so CoreSim (which predates these NoOps) can still run
        # the unsplit program from build_program()
        _split_multiwaits(_PROGRAM)
    res = bass_utils.run_bass_kernel_spmd(
        _PROGRAM, in_maps, core_ids=list(range(NCORE))
    )
    last_run_info["exec_time_ns"] = res.exec_time_ns
    last_run_info["trace"] = res.instructions_and_trace
    return assemble(X, res.results, gamma)
